# revision 15
# baseline (speedup 1.0000x reference)
"""Trainium2 kernel for nn_MultiHeadClassifier.

Math: out[i] = W[task_labels[i]] @ x[i] + b[task_labels[i]]
  x [262144, 1024] f32, task_labels [262144] int, W [8, 32, 1024], b [8, 32]

Strategy (8 NeuronCores, task-parallel):
  - Host sorts rows by task; core c processes (up to NCAP=32768) rows of
    task c, so W[c] is a per-core constant and there is NO routing on
    device at all — each core runs a plain GEMM. The ~few hundred rows
    that overflow a core's capacity are computed on host (numpy) and the
    result is merged back; bias is added on host.
  - x is sent as bf16 (host-side cast, halves the dominant HBM traffic;
    ~3e-3 relative error, well within tolerance), staged transposed
    ([NSB, 128, KO, SB]: superblock, d-within-ktile, ktile, row) so the
    PE contracts over d (partition dim) directly. 2 MB superblocks with
    16 KB contiguous per partition give near-peak DMA efficiency.
  - Per 512-row chunk: 8 accumulating matmuls with the W[c] k-tile as the
    32-column stationary operand and x as the 512-wide moving operand
    (bf16 = 1 cycle/col). PSUM [32, 512] f32 -> DVE copy w/ bf16 cast
    into a [128, 2048] output tile (4 chunks stacked on partitions,
    4 on free) -> one 512 KB output DMA per 16 chunks on the ACT ring
    (final group in 4 small pieces to shorten the tail).
  - DMA pace (~2 MB / superblock) keeps PE bursts ~3.5us with ~1.6us
    gaps, so the PE HAM clock gate stays warm and the x stream is the
    bottleneck: ~(67 MB + 2 MB) / ~390 GB/s + head/tail ~= 180 us.
"""

import sys

sys.path.insert(0, "/opt/trn_rl_repo")

import numpy as np
import ml_dtypes

import concourse.bass as bass
import concourse.tile as tile
from concourse import bacc, mybir
from concourse import bass_utils

B, D, C, T = 262144, 1024, 32, 8
NCORES = 8
P = 128
KO = D // P  # 8 contraction k-tiles
SB = 1024  # rows per superblock (one x DMA)
CH = 512  # rows per chunk (one PSUM accumulation group)
NSB = 32  # superblocks per core
NCAP = SB * NSB  # 32768 rows per core capacity
NCH = NCAP // CH  # 64 chunks
GRPC = 16  # chunks per output DMA group
NG = NCH // GRPC  # output groups

# set by test harness to collect a profile; harness-invoked kernel() keeps it off
TRACE = False
LAST_RESULTS = None
LAST_IN_MAPS = None


def _build():
    f32 = mybir.dt.float32
    bf16 = mybir.dt.bfloat16

    nc = bacc.Bacc("TRN2", debug=False, num_devices=NCORES)
    # xt[sb, ki, ko, r]: one superblock is a contiguous 2 MB region with
    # 16 KB contiguous per partition.
    xt_d = nc.dram_tensor("xt", [NSB, P, KO, SB], bf16, kind="ExternalInput")
    # wt[ko, ki, c] = W[core][c, ko*128+ki]
    wt_d = nc.dram_tensor("wt", [KO, P, C], bf16, kind="ExternalInput")
    out_d = nc.dram_tensor("out", [NG, P, (GRPC // 4) * CH], bf16, kind="ExternalOutput")

    with tile.TileContext(nc) as tc:
        with (
            tc.tile_pool(name="consts", bufs=1) as consts,
            tc.tile_pool(name="xpool", bufs=8) as xpool,
            tc.tile_pool(name="opool", bufs=2) as opool,
            tc.tile_pool(name="psum", bufs=6, space="PSUM") as psum,
        ):
            # first x superblock in flight before the consts
            xts0 = xpool.tile([P, KO, SB], bf16, tag="xts")
            nc.sync.dma_start(xts0[:], xt_d[0])

            # consts on the ACT ring: the SP ring stays a pure x stream
            wt = consts.tile([P, KO, C], bf16)
            nc.scalar.dma_start(wt[:], wt_d[:].rearrange("ko ki n -> ki ko n"))

            # Engine warmups: give PE and DVE one instruction that observes
            # the const DMA lane so steady-state instructions carry at most
            # one sync wait each.
            scratch = psum.tile([C, CH], f32, tag="y")
            nc.tensor.matmul(
                scratch[:2, :2], wt[:, 0, :2], wt[:, 0, :2], start=True, stop=True
            )
            dve_scr = consts.tile([1, C], bf16)
            nc.vector.tensor_copy(dve_scr[:], wt[:1, 0, :])

            for g in range(NG):
                last_g = g == NG - 1
                if not last_g:
                    out_g = opool.tile([P, (GRPC // 4) * CH], bf16, tag="out")
                for s in range(GRPC):
                    ch = g * GRPC + s
                    sb, c = ch // 2, ch % 2
                    if c == 0:
                        if sb == 0:
                            xts = xts0
                        else:
                            xts = xpool.tile([P, KO, SB], bf16, tag="xts")
                            nc.sync.dma_start(xts[:], xt_d[sb])
                    y = psum.tile([C, CH], f32, tag="y")
                    for ko in range(KO):
                        nc.tensor.matmul(
                            y[:],
                            wt[:, ko, :],
                            xts[:, ko, c * CH : (c + 1) * CH],
                            start=(ko == 0),
                            stop=(ko == KO - 1),
                        )
                    j, k = s % 4, s // 4
                    if last_g:
                        # final group: 4 small pieces so the tail DMA after
                        # the last matmul is ~128 KB, not 512 KB
                        if j == 0:
                            piece = opool.tile([P, CH], bf16, tag="piece")
                        nc.vector.tensor_copy(piece[C * j : C * (j + 1), :], y[:])
                        if j == 3:
                            nc.scalar.dma_start(
                                out_d[g, :, CH * k : CH * (k + 1)], piece[:]
                            )
                    else:
                        nc.vector.tensor_copy(
                            out_g[C * j : C * (j + 1), CH * k : CH * (k + 1)], y[:]
                        )
                if not last_g:
                    # out on the ACT HWDGE ring so it never delays xts loads
                    # queued on the SP ring
                    nc.scalar.dma_start(out_d[g], out_g[:])
    nc.compile()
    return nc


_NC = None


def _get_nc():
    global _NC
    if _NC is None:
        _NC = _build()
    return _NC


def kernel(x, task_labels, W, b):
    global LAST_RESULTS, LAST_IN_MAPS
    x = np.asarray(x)
    if x.dtype != np.float32:
        x = x.astype(np.float32)
    labels = np.asarray(task_labels).astype(np.int64)
    W = np.asarray(W)
    if W.dtype != np.float32:
        W = W.astype(np.float32)
    b = np.asarray(b)
    if b.dtype != np.float32:
        b = b.astype(np.float32)

    order = np.argsort(labels, kind="stable")  # rows grouped by task
    counts = np.bincount(labels, minlength=T)
    starts = np.concatenate([[0], np.cumsum(counts)])

    in_maps = []
    over_rows = []  # (task, global row indices beyond capacity)
    for t in range(T):
        seg_idx = order[starts[t] : starts[t + 1]]
        n_dev = min(counts[t], NCAP)
        xs = np.zeros((NCAP, D), dtype=ml_dtypes.bfloat16)
        xs[:n_dev] = x[seg_idx[:n_dev]]
        # xt[sb, ki, ko, r] = xs[sb*SB + r, ko*P + ki]
        xt = np.ascontiguousarray(xs.reshape(NSB, SB, KO, P).transpose(0, 3, 2, 1))
        wt = np.ascontiguousarray(W[t].T.reshape(KO, P, C)).astype(ml_dtypes.bfloat16)
        in_maps.append({"xt": xt, "wt": wt})
        if counts[t] > NCAP:
            over_rows.append((t, seg_idx[NCAP:]))

    LAST_IN_MAPS = in_maps
    nc = _get_nc()
    res = bass_utils.run_bass_kernel_spmd(
        nc, in_maps, core_ids=list(range(NCORES)), trace=TRACE
    )
    LAST_RESULTS = res

    out = np.empty((B, C), dtype=np.float32)
    for t in range(T):
        seg_idx = order[starts[t] : starts[t + 1]]
        n_dev = min(counts[t], NCAP)
        # out_d[g, 4(j) 32(c), 4(k) 512(r)] chunk = g*16 + k*4 + j, rows 512*ch
        o = np.asarray(res.results[t]["out"]).astype(np.float32)
        o = o.reshape(NG, 4, C, GRPC // 4, CH).transpose(0, 3, 1, 4, 2).reshape(NCAP, C)
        out[seg_idx[:n_dev]] = o[:n_dev]
    for t, idx in over_rows:
        out[idx] = x[idx] @ W[t].T
    out += b[labels]
    return out
